# revision 12
# baseline (speedup 1.0000x reference)
"""Trainium2 Bass kernel: windowed 32-pt FFT -> top-8 magnitude mask -> iFFT.

Per core (pure data parallel over batch), 64 tiles of [128, 512] fp32,
freq-major input (partition 32g+n = time sample n of row-group g):
  PE  : Bm matmul (fp32 half-spectrum windowed DFT, exact so the top-8
        selection basis matches the reference bit-for-bit);
        4x per tile "PmT" matmuls with the SQUARED spectrum as the
        stationary operand and the 0/1 conjugate-pair-add matrix as the
        256-wide f32r moving operand -- this emits |X|^2 directly in
        ROW-major layout (rows on partitions), so no magnitude transpose
        is ever needed; Cm reconstruction matmul (fp16 x fp16).
  ACT : square (f32r out, the only lossy step: RNE to 11 mantissa bits),
        sqrt (PSUM->SBUF), final fp16 out-copy.
  Pool: tie-break bias multiply by the sqrt(1 - k*eta) free-dim pattern.
  DVE : 16x sorted-top-8 InstMax per tile (the irreducible core of the
        top-k), one fused custom op coef = (mag >= th8) ? mag : 0, and
        one 32x32-block transpose back to freq-major per half-tile
        (valid because Cm is 32-block-diagonal; the host post-permute
        absorbs the block-row reordering).

A 1-deep software pipeline keeps the DVE FIFO from stalling: pair j's
mask/reconstruction issues after pair j+1's InstMax batch. The bias makes
otherwise bitwise-equal conjugate-pair magnitudes strictly decreasing in
k, so ">= 8th largest" selects exactly 8 entries per row, ties broken
toward lower k like jax.lax.top_k; Cm folds the inverse bias. Output is
fp16 (halves out-DMA); the host converts back to fp32.
"""

import math

import numpy as np

B_TOTAL = 1048576
S = 32
N_CORES = 8
R_PER_CORE = B_TOTAL // N_CORES  # 131072
TILE_F = 512
ROWS_PER_TILE = 4 * TILE_F       # 2048
N_TILES = R_PER_CORE // ROWS_PER_TILE  # 64
SEGS = TILE_F // 32              # 16
ETA = 2.0 ** -20

_cache = {}


def _build_consts():
    n = np.arange(S, dtype=np.float64)
    w = (0.5 - 0.5 * np.cos(2.0 * np.pi * np.arange(S, dtype=np.float32) / S))
    w = w.astype(np.float32).astype(np.float64)

    B32 = np.zeros((S, S), dtype=np.float64)
    for m in range(17):
        B32[:, m] = w * np.cos(2.0 * np.pi * m * n / S)
    for j in range(1, 16):
        B32[:, 16 + j] = -w * np.sin(2.0 * np.pi * j * n / S)

    c = 1.0 - np.arange(S, dtype=np.float64) * ETA

    # pure 0/1 pair-add matrix — exact under f32r rounding
    Pm = np.zeros((S, S), dtype=np.float64)
    for kk in range(S):
        j = min(kk, S - kk)
        Pm[j, kk] = 1.0
        if 1 <= j <= 15:
            Pm[16 + j, kk] = 1.0

    Cm = np.zeros((S, S), dtype=np.float64)
    for kk in range(S):
        Cm[kk, :] = np.cos(2.0 * np.pi * kk * n / S) / (S * math.sqrt(c[kk]))

    def blockdiag4(M):
        out = np.zeros((128, 128), dtype=np.float32)
        for g in range(4):
            out[g * 32:(g + 1) * 32, g * 32:(g + 1) * 32] = M.astype(np.float32)
        return out

    bm = blockdiag4(B32)
    # 0/1 pair-add, duplicated to 256 cols: f32r matmuls need a moving
    # free dim >= 256 to hit the 1 cyc/row fast path
    pm1 = blockdiag4(Pm)
    pm = np.concatenate([pm1, pm1], axis=1)   # [128, 256], f32r-exact
    cm = blockdiag4(Cm).astype(np.float16)
    idb = np.eye(128, dtype=np.float16)
    # row-major bias pattern: sqrt(c_k) at col 32*t + k, materialized
    # across the full 512-wide tile
    scp = np.tile(np.sqrt(c).astype(np.float32), (128, 16))  # [128, 512]
    return bm, pm, cm, scp, idb


def _register_select_ge():
    """Fused coef = (mag >= th) ? mag : 0 as one custom DVE op."""
    import numpy as np
    import concourse.dve_ops as dve_ops
    from concourse.dve_spec import Spec, Src0, Src1, Zero, select, lower
    from concourse.dve_uop import DveOpSpec

    name = "SELECT_GE_ANT"
    if name in dve_ops._SUB_OPCODE_FOR_NAME:
        return next(op for op in dve_ops.OPS if op.name == name)
    spec = Spec(
        body=select(Src0 >= Src1, Src0, Zero),
        reference=lambda in0, in1, c0, c1, c2: np.where(
            in0.reshape(in0.shape[0], -1)
            >= np.ascontiguousarray(in1).reshape(in1.shape[0], -1),
            in0.reshape(in0.shape[0], -1), 0.0,
        ),
    )
    op = dve_ops.DveOp(name, spec, subdim=False, uops_sha={})
    dve_ops.OPS.append(op)
    dve_ops._SUB_OPCODE_FOR_NAME[name] = (
        dve_ops._CUSTOM_DVE_ROW_BASE + len(dve_ops.OPS) - 1
    )
    dve_ops.CUSTOM_DVE_SPECS[name] = spec
    for ver in ("v3", "v4"):
        tmp = DveOpSpec(
            name=name,
            opcode=dve_ops.get_dve_sub_opcode(name),
            uops=lower(spec, ver=ver),
            rd1_en=True,
        )
        op.uops_sha[ver] = tmp.sha(ver)
    return op


def _build_program():
    import concourse.mybir as mybir
    from concourse import bacc
    from concourse.tile import TileContext

    sel_op = _register_select_ge()

    f32 = mybir.dt.float32
    f32r = mybir.dt.float32r
    f16 = mybir.dt.float16
    nc = bacc.Bacc("TRN2", target_bir_lowering=False, debug=False)

    x_d = nc.dram_tensor("x", [N_TILES, 128, TILE_F], f32, kind="ExternalInput")
    bm_d = nc.dram_tensor("Bm", [128, 128], f32, kind="ExternalInput")
    pm_d = nc.dram_tensor("Pm", [128, 256], f32r, kind="ExternalInput")
    cm_d = nc.dram_tensor("Cm", [128, 128], f16, kind="ExternalInput")
    sc_d = nc.dram_tensor("Sc", [128, TILE_F], f32, kind="ExternalInput")
    idb_d = nc.dram_tensor("Idb", [128, 128], f16, kind="ExternalInput")
    out_d = nc.dram_tensor("out", [N_TILES, 128, TILE_F], f16,
                           kind="ExternalOutput")

    x_v = x_d.ap()
    out_v = out_d.ap()

    with TileContext(nc) as tc:
        with (
            tc.tile_pool(name="consts", bufs=1) as cpool,
            tc.tile_pool(name="io", bufs=6) as io_pool,
            tc.tile_pool(name="work", bufs=5) as work_pool,
            tc.tile_pool(name="psA", bufs=2, space="PSUM") as psA,
            tc.tile_pool(name="psB", bufs=2, space="PSUM") as psB,
            tc.tile_pool(name="psO", bufs=1, space="PSUM") as psO,
            tc.tile_pool(name="psT", bufs=1, space="PSUM") as psT,
        ):
            bm = cpool.tile([128, 128], f32, tag="bm")
            pm = cpool.tile([128, 256], f32r, tag="pm")
            cm = cpool.tile([128, 128], f16, tag="cm")
            sc = cpool.tile([128, TILE_F], f32, tag="sc")
            idb = cpool.tile([128, 128], f16, tag="idb")
            # first pair's inputs start streaming before the consts so the
            # front-end pipeline fills as early as possible
            x_pre = []
            for i in (0, 1):
                x_t = io_pool.tile([128, TILE_F], f32, tag="x_t")
                nc.sync.dma_start(x_t[:], x_v[i])
                x_pre.append(x_t)
            nc.sync.dma_start(bm[:], bm_d.ap())
            nc.sync.dma_start(pm[:], pm_d.ap())
            nc.sync.dma_start(sc[:], sc_d.ap())
            nc.sync.dma_start(cm[:], cm_d.ap())
            nc.sync.dma_start(idb[:], idb_d.ap())

            W = 2 * TILE_F
            SEG2 = 2 * SEGS

            def frontend(j):
                mag_rm = work_pool.tile([128, W], f32, tag="mag_rm")
                for h in (0, 1):
                    i = 2 * j + h
                    if j == 0:
                        x_t = x_pre[h]
                    else:
                        x_t = io_pool.tile([128, TILE_F], f32, tag="x_t")
                        nc.sync.dma_start(x_t[:], x_v[i])

                    g_ps = psA.tile([128, TILE_F], f32, tag="g")
                    nc.tensor.matmul(g_ps[:], bm[:], x_t[:],
                                     start=True, stop=True)

                    sq = work_pool.tile([128, TILE_F], f32r, tag="sq")
                    nc.scalar.square(sq[:], g_ps[:])

                    # data-as-stationary: s_rm[i, 32g+k] per 128-row block.
                    # 256-wide moving (pm duplicated) for the f32r fast
                    # path; only the first 128 cols of each block are used.
                    s_ps = psB.tile([128, 2 * TILE_F], f32, tag="s")
                    for b in range(4):
                        nc.tensor.matmul(
                            s_ps[:, 256 * b:256 * (b + 1)],
                            sq[:, 128 * b:128 * (b + 1)], pm[:],
                            start=True, stop=True,
                        )

                    magh = work_pool.tile([128, TILE_F], f32, tag="magh")
                    s_used = s_ps[:].rearrange(
                        "p (b f) -> p b f", f=256)[:, :, 0:128]
                    nc.scalar.sqrt(magh[:].rearrange(
                        "p (b f) -> p b f", f=128), s_used)

                    # bias multiply: mag_rm = magh * sqrt(c) pattern
                    nc.gpsimd.tensor_mul(
                        mag_rm[:, TILE_F * h:TILE_F * (h + 1)],
                        magh[:], sc[:],
                    )

                th8 = work_pool.tile([128, 8 * SEG2], f32, tag="th8")
                for t in range(SEG2):
                    nc.vector.max(
                        out=th8[:, 8 * t:8 * t + 8],
                        in_=mag_rm[:, 32 * t:32 * t + 32],
                    )
                return mag_rm, th8

            def backend(j, mag_rm, th8):
                coef_rm = work_pool.tile([128, W], f16, tag="coef_rm")
                coef_t = work_pool.tile([128, W], f16, tag="coef_t")
                for h in (0, 1):
                    i = 2 * j + h
                    sl = slice(TILE_F * h, TILE_F * (h + 1))
                    th_b = th8[:, 8 * SEGS * h + 7:8 * SEGS * (h + 1):8] \
                        .to_broadcast([128, SEGS, 32])
                    mag3 = mag_rm[:, sl].rearrange("p (t n) -> p t n", n=32)
                    coef3 = coef_rm[:, sl].rearrange("p (t n) -> p t n", n=32)
                    # fused (mag >= th) ? mag : 0 in one DVE op
                    nc.vector._custom_dve(
                        sel_op, out=coef3, in0=mag3, in1=th_b
                    )
                    # PE full 128-block transposes (fp16 transpose mode)
                    # -> baseline freq-major layout; ACT copies PSUM->SBUF
                    coefT_ps = psT.tile([128, TILE_F], f16, tag="coefT")
                    for b in range(4):
                        nc.tensor.transpose(
                            coefT_ps[:, 128 * b:128 * (b + 1)],
                            coef_rm[:, TILE_F * h + 128 * b:
                                    TILE_F * h + 128 * (b + 1)],
                            idb[:],
                        )
                    nc.scalar.copy(coef_t[:, sl], coefT_ps[:])

                    o_ps = psO.tile([128, TILE_F], f32, tag="o")
                    nc.tensor.matmul(
                        o_ps[:], cm[:], coef_t[:, sl],
                        start=True, stop=True,
                    )

                    o_sb = io_pool.tile([128, TILE_F], f16, tag="o_sb")
                    nc.scalar.copy(o_sb[:], o_ps[:])

                    nc.sync.dma_start(out_v[i], o_sb[:])

            # 1-deep software pipeline: pair j's backend issues after pair
            # j+1's InstMax batch, so the DVE FIFO never stalls on Pool.
            pending = None
            for j in range(N_TILES // 2):
                state = frontend(j)
                if pending is not None:
                    backend(j - 1, *pending)
                pending = state
            backend(N_TILES // 2 - 1, *pending)

    nc.compile()
    return nc


def _get_program():
    if "nc" not in _cache:
        _cache["nc"] = _build_program()
        _cache["consts"] = _build_consts()
    return _cache["nc"], _cache["consts"]


def _pre_permute(xc: np.ndarray) -> np.ndarray:
    t = xc.reshape(N_TILES, 4, TILE_F, S)          # [i, g, f, n]
    return np.ascontiguousarray(t.transpose(0, 1, 3, 2)).reshape(
        N_TILES, 128, TILE_F
    )


def _post_permute(op: np.ndarray) -> np.ndarray:
    # baseline freq-major: out[i, 32g+n, f] = rec(row 2048 i + 512 g + f, n)
    t = op.astype(np.float32).reshape(N_TILES, 4, S, TILE_F)
    t = t.transpose(0, 1, 3, 2)                  # [i, g, f, n]
    return np.ascontiguousarray(t).reshape(R_PER_CORE, S)


def _const_map():
    nc, (bm, pm, cm, scp, idb) = _get_program()
    return {"Bm": bm, "Pm": pm, "Cm": cm, "Sc": scp, "Idb": idb}


def kernel(x: np.ndarray) -> np.ndarray:
    from concourse.bass_utils import run_bass_kernel_spmd

    nc, _ = _get_program()
    consts = _const_map()

    xc = np.ascontiguousarray(x[:, :, 0], dtype=np.float32)  # [B, 32]
    shards = xc.reshape(N_CORES, R_PER_CORE, S)
    in_maps = [
        {"x": _pre_permute(shards[c]), **consts}
        for c in range(N_CORES)
    ]
    res = run_bass_kernel_spmd(nc, in_maps, core_ids=list(range(N_CORES)))
    out = np.concatenate(
        [_post_permute(r["out"]) for r in res.results], axis=0
    )
    return out.reshape(B_TOTAL, S, 1).astype(np.float32)

